# revision 1
# baseline (speedup 1.0000x reference)
"""Trainium2 Bass kernel: per-batch segment-mean pooling + 3-layer MLP.

Reference computation (B=64, T=512, H=768, S=128):
  pooled[b,s,:] = mean over t of hidden[b,t,:] where statements_ids[b,t]==s
  x = gelu(pooled @ w1 + b1); x = gelu(x @ w2 + b2)
  out[b,s] = sigmoid(x @ w3 + b3)

Distribution: data-parallel over batch across 8 NeuronCores (8 batches per
core); MLP weights replicated.

Per-core algorithm (all matmuls on PE at fp32r 1 cycle/row):
  - Build the one-hot matrix MT[t,s] = (sid[t]==s) on DVE via
    tensor_scalar(is_equal) against an iota constant.
  - counts = MT.T @ ones            (PE)        -> inv = 1/max(counts,1) (DVE)
  - pooled_sums = MT.T @ hidden[b]  (PE, [S,H]) -> pooled = sums*inv     (DVE)
  - X^T tiles via PE transpose (pooled is [S,H] but the MLP wants [H, rows])
  - MLP batched over all 8 local batches: rows = 8*128 = 1024 moving dim,
    weights stationary; gelu/sigmoid + bias fused on ACT.
"""

import os
import sys

sys.path.insert(0, "/opt/trn_rl_repo")

import numpy as np

import concourse.bass as bass
import concourse.mybir as mybir
import concourse.tile as tile
from concourse import bacc, bass_utils

B, T, H, S = 64, 512, 768, 128
N_CORES = 8
BL = B // N_CORES  # local batches per core
P = 128
KT = T // P        # t-tiles per batch
KH = H // P        # h-tiles
R = BL * S         # MLP rows per core
RC = 2 * S         # moving-dim chunk (2 batches) -- >=256 keeps fp32r at 1 cyc/row
NRC = R // RC
HF = H + 2         # hidden padded with 2 constant 1.0 columns (counts trick)
CR_COLS = 134      # f32r packed consts (matmul operands): ident | w3
CF_COLS = 173      # f32 packed consts: iota | sid-bits | b1 | b2 | b3

_CACHE: dict = {}


def _build_program(act_func=None):
    f32, f32r, i32 = mybir.dt.float32, mybir.dt.float32r, mybir.dt.int32
    FT = mybir.ActivationFunctionType
    OP = mybir.AluOpType

    nc = bacc.Bacc("TRN2", target_bir_lowering=False, debug=False)
    hid = nc.dram_tensor("hidden", [BL, T, HF], f32r, kind="ExternalInput").ap()
    w1 = nc.dram_tensor("w1", [H, H], f32r, kind="ExternalInput").ap()
    w2 = nc.dram_tensor("w2", [H, H], f32r, kind="ExternalInput").ap()
    cpack_r = nc.dram_tensor("cpack_r", [P, CR_COLS], f32r, kind="ExternalInput").ap()
    cpack_f = nc.dram_tensor("cpack_f", [P, CF_COLS], f32, kind="ExternalInput").ap()
    out = nc.dram_tensor("out", [BL, S], f32, kind="ExternalOutput").ap()

    with tile.TileContext(nc) as tc:
        with (
            tc.tile_pool(name="consts", bufs=1) as consts,
            tc.tile_pool(name="wpool", bufs=1) as wpool,
            tc.tile_pool(name="hpool", bufs=1) as hpool,
            tc.tile_pool(name="mtpool", bufs=8) as mtpool,
            tc.tile_pool(name="small", bufs=3) as small,
            tc.tile_pool(name="xtpool", bufs=1) as xtpool,
            tc.tile_pool(name="ypool", bufs=1) as ypool,
            tc.tile_pool(name="ps", bufs=8, space="PSUM") as ps,
        ):
            # ---- all small constants arrive in ONE packed DMA (single
            # 1.2KB line per partition) so the hidden stream starts at once ----
            cpf_sb = consts.tile([P, CF_COLS], f32)
            nc.sync.dma_start(cpf_sb, cpack_f)
            cpr_sb = consts.tile([P, CR_COLS], f32r)
            nc.sync.dma_start(cpr_sb, cpack_r)
            ident_sb = cpr_sb[:, 0:P]
            w3_sb = cpr_sb[:, P : P + KH]
            iota_sb = cpf_sb[:, 0:P]
            sid_sb = cpf_sb[:, P : P + BL * KT].bitcast(i32)
            b1_sb = cpf_sb[:, 160:166]
            b2_sb = cpf_sb[:, 166:172]
            b3_sb = cpf_sb[0:1, 172:173]

            # ---- hidden + weight streaming on sync/HWDGE, ordered to match
            # the compute pipeline: hidden batches pace the pooling; weight
            # k-tiles trickle between batches so fc1/fc2 unlock per-k ----
            hbs = [None] * BL
            w1ks = [None] * KH
            w2ks = [None] * KH

            def load_hb(b):
                if b < 2:
                    # first two batches arrive per k-chunk so pooling starts
                    # on the first 0.4 MB instead of the full 1.6 MB batch
                    tiles = []
                    for k in range(KT):
                        t = hpool.tile([P, HF], f32r, tag=f"hb{b}k{k}", name=f"hb{b}k{k}")
                        nc.sync.dma_start(t, hid[b, k * P : (k + 1) * P, :])
                        tiles.append(t)
                    hbs[b] = tiles
                else:
                    hb = hpool.tile(
                        [P, KT, HF], f32r, tag=f"hb{2 + (b - 2) % 3}", name=f"hb{b}"
                    )
                    nc.sync.dma_start(hb, hid[b].rearrange("(k p) h -> p k h", p=P))
                    hbs[b] = hb

            def hb_slice(b, k, lo, hi):
                if b < 2:
                    return hbs[b][k][:, lo:hi]
                return hbs[b][:, k, lo:hi]

            def load_w(ws, wdram, k, nm):
                ws[k] = wpool.tile([P, H], f32r, tag=f"{nm}{k}", name=f"{nm}{k}")
                nc.sync.dma_start(ws[k], wdram[k * P : (k + 1) * P, :])

            load_hb(0)
            for k in range(3):
                load_w(w1ks, w1, k, "w1k")
            load_hb(1)
            for k in range(3, KH):
                load_w(w1ks, w1, k, "w1k")
            load_hb(2)
            load_hb(3)
            for k in range(KH):
                load_w(w2ks, w2, k, "w2k")
            load_hb(4)
            load_hb(5)
            load_hb(6)
            load_hb(7)

            xts = [xtpool.tile([P, R], f32r, tag=f"xt{k}", name=f"xt{k}") for k in range(KH)]
            y1s = [ypool.tile([P, R], f32r, tag=f"y1_{m}", name=f"y1_{m}") for m in range(KH)]
            y2s = [ypool.tile([P, R], f32r, tag=f"y2_{m}", name=f"y2_{m}") for m in range(KH)]
            pred = ypool.tile([1, R], f32, tag="pred")

            C0 = 512          # pooling psum chunk 0: cols [0, 512)
            C1 = HF - C0      # chunk 1: cols [512, 770) -- col 768 = counts

            pooleds = [None] * BL

            def pool_mm(b):
                sidf = small.tile([P, KT], f32, tag="sidf")
                nc.vector.tensor_copy(sidf, sid_sb[:, b * KT : (b + 1) * KT])
                mts = []
                for k in range(KT):
                    mt = mtpool.tile([P, P], f32r, tag="mt")
                    nc.vector.tensor_tensor(
                        mt,
                        iota_sb,
                        sidf[:, k : k + 1].to_broadcast((P, P)),
                        OP.is_equal,
                    )
                    mts.append(mt)
                # counts chunk first so the inv chain runs while pp0 matmuls
                pp1 = ps.tile([P, C1], f32, tag="ps")
                pp0 = ps.tile([P, C0], f32, tag="ps")
                # interleave the two accumulation groups per k-chunk: both
                # matmuls of an arrived chunk fire at once instead of pp1(k3)
                # blocking ready pp0 work in the in-order PE stream
                for k in range(KT):
                    nc.tensor.matmul(
                        pp1, lhsT=mts[k], rhs=hb_slice(b, k, C0, HF),
                        start=(k == 0), stop=(k == KT - 1),
                    )
                    nc.tensor.matmul(
                        pp0, lhsT=mts[k], rhs=hb_slice(b, k, 0, C0),
                        start=(k == 0), stop=(k == KT - 1),
                    )
                inv = small.tile([P, 1], f32, tag="inv")
                nc.vector.tensor_scalar(inv, pp1[:, H - C0 : H - C0 + 1], 1.0, None, OP.max)
                nc.vector.reciprocal(inv, inv)
                pooled = small.tile([P, H], f32r, tag="pooled")
                # normalize in transpose-consumption order, smallest first:
                # [0:128] unblocks transpose m0 immediately, [128:512] covers
                # m1-m3 while m0 runs, [512:768] covers m4-m5
                nc.vector.tensor_tensor(
                    pooled[:, 0:P], pp0[:, 0:P], inv[:, 0:1].to_broadcast((P, P)),
                    OP.mult,
                )
                nc.vector.tensor_tensor(
                    pooled[:, P:C0], pp0[:, P:C0],
                    inv[:, 0:1].to_broadcast((P, C0 - P)), OP.mult,
                )
                nc.vector.tensor_tensor(
                    pooled[:, C0:H], pp1[:, 0 : H - C0],
                    inv[:, 0:1].to_broadcast((P, H - C0)), OP.mult,
                )
                pooleds[b] = pooled

            def pool_tr(b):
                pooled = pooleds[b]
                for m in range(KH):
                    trp = ps.tile([P, P], f32r, tag="ps")
                    nc.tensor.transpose(trp, pooled[:, m * P : (m + 1) * P], ident_sb)
                    nc.vector.tensor_copy(xts[m][:, b * S : (b + 1) * S], trp)

            def fc(wks, b_sb, xs, outs, rc, func):
                for m in range(KH):
                    pt = ps.tile([P, RC], f32, tag="ps")
                    for k in range(KH):
                        nc.tensor.matmul(
                            pt,
                            lhsT=wks[k][:, m * P : (m + 1) * P],
                            rhs=xs[k][:, rc * RC : (rc + 1) * RC],
                            start=(k == 0),
                            stop=(k == KH - 1),
                        )
                    nc.scalar.activation(
                        outs[m][:, rc * RC : (rc + 1) * RC],
                        pt,
                        func,
                        bias=b_sb[:, m : m + 1],
                    )

            def fc3(rc):
                pt = ps.tile([1, RC], f32, tag="ps")
                for k in range(KH):
                    nc.tensor.matmul(
                        pt,
                        lhsT=w3_sb[:, k : k + 1],
                        rhs=y2s[k][:, rc * RC : (rc + 1) * RC],
                        start=(k == 0),
                        stop=(k == KH - 1),
                    )
                nc.scalar.activation(
                    pred[:, rc * RC : (rc + 1) * RC],
                    pt,
                    mybir.ActivationFunctionType.Sigmoid,
                    bias=b3_sb,
                )
                # stream this chunk's predictions out immediately; only the
                # final 1 KB remains on the critical path after the last sigmoid
                nc.sync.dma_start(
                    out.rearrange("b s -> (b s)")[rc * RC : (rc + 1) * RC],
                    pred[:, rc * RC : (rc + 1) * RC],
                )

            FT = mybir.ActivationFunctionType
            gelu = FT.Gelu if act_func is None else act_func
            pool_mm(0)
            pool_tr(0)
            pool_mm(1)
            pool_tr(1)
            fc(w1ks, b1_sb, xts, y1s, 0, gelu)
            pool_mm(2)
            pool_tr(2)
            pool_mm(3)
            pool_tr(3)
            fc(w1ks, b1_sb, xts, y1s, 1, gelu)
            fc(w2ks, b2_sb, y1s, y2s, 0, gelu)
            fc3(0)
            pool_mm(4)
            pool_tr(4)
            pool_mm(5)
            pool_tr(5)
            fc(w1ks, b1_sb, xts, y1s, 2, gelu)
            fc(w2ks, b2_sb, y1s, y2s, 1, gelu)
            fc3(1)
            pool_mm(6)
            pool_tr(6)
            pool_mm(7)
            pool_tr(7)
            fc(w1ks, b1_sb, xts, y1s, 3, gelu)
            fc(w2ks, b2_sb, y1s, y2s, 2, gelu)
            fc3(2)
            fc(w2ks, b2_sb, y1s, y2s, 3, gelu)
            fc3(3)

    nc.compile()
    return nc


def _get_program():
    if "nc" not in _CACHE:
        _CACHE["nc"] = _build_program()
    return _CACHE["nc"]


def _cpack(sid_shard, b1, b2, b3, w3):
    """Pack per-core constants into two tensors: f32r (matmul operands,
    the DMA may round these) and plain f32 (bit-exact: iota, sid bits,
    biases)."""
    cr = np.zeros((P, CR_COLS), dtype=np.float32)
    cr[:, 0:P] = np.eye(P, dtype=np.float32)
    cr[:, P : P + KH] = np.asarray(w3, np.float32).reshape(KH, P, 1)[:, :, 0].T
    cf = np.zeros((P, CF_COLS), dtype=np.float32)
    cf[:, 0:P] = np.arange(P, dtype=np.float32)[None, :]
    sid_cols = np.transpose(
        sid_shard.astype(np.int32).reshape(BL, KT, P), (2, 0, 1)
    ).reshape(P, BL * KT)
    cf[:, P : P + BL * KT] = sid_cols.view(np.float32)
    cf[:, 160:166] = np.asarray(b1, np.float32).reshape(KH, P).T
    cf[:, 166:172] = np.asarray(b2, np.float32).reshape(KH, P).T
    cf[0, 172] = np.float32(np.asarray(b3).reshape(-1)[0])
    return cr, cf


def make_in_maps(hidden, statements_ids, w1, b1, w2, b2, w3, b3):
    hidden = np.asarray(hidden, dtype=np.float32)
    pad = np.ones((*hidden.shape[:2], HF - H), dtype=np.float32)
    hidden = np.ascontiguousarray(np.concatenate([hidden, pad], axis=-1))
    sid = np.asarray(statements_ids, dtype=np.int32)
    w1 = np.ascontiguousarray(np.asarray(w1, dtype=np.float32))
    w2 = np.ascontiguousarray(np.asarray(w2, dtype=np.float32))
    in_maps = []
    for c in range(N_CORES):
        cr, cf = _cpack(sid[c * BL : (c + 1) * BL], b1, b2, b3, w3)
        in_maps.append(
            {
                "hidden": hidden[c * BL : (c + 1) * BL],
                "w1": w1,
                "w2": w2,
                "cpack_r": cr,
                "cpack_f": cf,
            }
        )
    return in_maps


def kernel(hidden, statements_ids, w1, b1, w2, b2, w3, b3, **kwargs):
    nc = _get_program()
    in_maps = make_in_maps(hidden, statements_ids, w1, b1, w2, b2, w3, b3)
    trace = bool(int(os.environ.get("KERNEL_TRACE", "0")))
    res = bass_utils.run_bass_kernel_spmd(
        nc, in_maps, core_ids=list(range(N_CORES)), trace=trace
    )
    _CACHE["last_results"] = res
    out = np.concatenate([res.results[c]["out"] for c in range(N_CORES)], axis=0)
    return out.astype(np.float32)



# revision 2
# speedup vs baseline: 1.0613x; 1.0613x over previous
"""Trainium2 Bass kernel: per-batch segment-mean pooling + 3-layer MLP.

Reference computation (B=64, T=512, H=768, S=128):
  pooled[b,s,:] = mean over t of hidden[b,t,:] where statements_ids[b,t]==s
  x = gelu(pooled @ w1 + b1); x = gelu(x @ w2 + b2)
  out[b,s] = sigmoid(x @ w3 + b3)

Distribution: data-parallel over batch across 8 NeuronCores (8 batches per
core); MLP weights replicated.

v2 design notes (PE streams at N cols/cycle regardless of dtype, so the
wins are bandwidth + gap-closing, not matmul cycles):
  - All tensor streams in bf16: halves HBM traffic (9.75 MB/core vs 17.3)
    so the pooling phase is no longer DMA-paced.
  - The one-hot matrix MT[t,s] (exact 0/1 values in bf16) is built on the
    host and DMA'd in: removes the iota/is_equal/cast DVE chain from the
    critical path at startup.
  - counts -> inv = 1/max(counts,1) computed on host (f32, exact): kills
    the padded ones-columns, the counts matmul and the max/reciprocal
    chain; pooling psum is evacuated with a single f32-psum * inv
    broadcast multiply (DVE) per chunk, writing bf16.
  - All 8 hidden batches are SBUF-resident (bf16 halves footprint) so
    every DMA is issued up front and streams at full rate.
  - Sigmoids are deferred and batched (2 ACT table switches instead of 8;
    each 1.28us): rc0-2 sigmoid mid-run under pool(6/7), only rc3's
    sigmoid (+4KB out DMA) sits on the tail behind the last gelu.
"""

import os
import sys

sys.path.insert(0, "/opt/trn_rl_repo")

import ml_dtypes
import numpy as np

import concourse.bass as bass
import concourse.mybir as mybir
import concourse.tile as tile
from concourse import bacc, bass_utils

B, T, H, S = 64, 512, 768, 128
N_CORES = 8
BL = B // N_CORES  # local batches per core
P = 128
KT = T // P        # t-tiles per batch
KH = H // P        # h-tiles
R = BL * S         # MLP rows per core
RC = 2 * S         # moving-dim chunk (2 batches)
NRC = R // RC
MTC = BL * KT * S  # packed one-hot columns
CH_COLS = P + KH           # bf16 packed consts: ident | w3
CF_COLS = BL + 2 * KH + 1  # f32 packed consts: inv | b1 | b2 | b3

BF16 = ml_dtypes.bfloat16

_CACHE: dict = {}


def _build_program():
    f32, bf16 = mybir.dt.float32, mybir.dt.bfloat16
    FT = mybir.ActivationFunctionType
    OP = mybir.AluOpType

    nc = bacc.Bacc("TRN2", target_bir_lowering=False, debug=False)
    hid = nc.dram_tensor("hidden", [BL, T, H], bf16, kind="ExternalInput").ap()
    mtn = nc.dram_tensor("mtn", [P, MTC], bf16, kind="ExternalInput").ap()
    w1 = nc.dram_tensor("w1", [H, H], bf16, kind="ExternalInput").ap()
    w2 = nc.dram_tensor("w2", [H, H], bf16, kind="ExternalInput").ap()
    cpack_h = nc.dram_tensor("cpack_h", [P, CH_COLS], bf16, kind="ExternalInput").ap()
    cpack_f = nc.dram_tensor("cpack_f", [P, CF_COLS], f32, kind="ExternalInput").ap()
    out = nc.dram_tensor("out", [BL, S], f32, kind="ExternalOutput").ap()

    with tile.TileContext(nc) as tc:
        with (
            tc.tile_pool(name="consts", bufs=1) as consts,
            tc.tile_pool(name="wpool", bufs=1) as wpool,
            tc.tile_pool(name="hpool", bufs=1) as hpool,
            tc.tile_pool(name="small", bufs=3) as small,
            tc.tile_pool(name="xtpool", bufs=1) as xtpool,
            tc.tile_pool(name="ypool", bufs=1) as ypool,
            tc.tile_pool(name="ps", bufs=8, space="PSUM") as ps,
        ):
            # ---- constants first (small, unblock nothing heavy) ----
            cph_sb = consts.tile([P, CH_COLS], bf16)
            nc.sync.dma_start(cph_sb, cpack_h)
            cpf_sb = consts.tile([P, CF_COLS], f32)
            nc.sync.dma_start(cpf_sb, cpack_f)
            ident_sb = cph_sb[:, 0:P]
            w3_sb = cph_sb[:, P : P + KH]
            inv_sb = cpf_sb[:, 0:BL]
            b1_sb = cpf_sb[:, BL : BL + KH]
            b2_sb = cpf_sb[:, BL + KH : BL + 2 * KH]
            b3_sb = cpf_sb[0:1, BL + 2 * KH : BL + 2 * KH + 1]

            # one-hot pack: batches 0-1 first (pooling starts on them)
            mtn_sb = consts.tile([P, MTC], bf16)
            nc.sync.dma_start(mtn_sb[:, 0 : 2 * KT * S], mtn[:, 0 : 2 * KT * S])

            # ---- hidden + weight streaming; everything SBUF-resident so
            # DMA free-runs; issue order == consumption order ----
            hbs = [None] * BL
            w1ks = [None] * KH
            w2ks = [None] * KH

            def load_hb(b):
                if b < 2:
                    # first two batches arrive per k-chunk so pooling can
                    # start on the first 0.2 MB
                    tiles = []
                    for k in range(KT):
                        t = hpool.tile([P, H], bf16, tag=f"hb{b}k{k}", name=f"hb{b}k{k}")
                        nc.sync.dma_start(t, hid[b, k * P : (k + 1) * P, :])
                        tiles.append(t)
                    hbs[b] = tiles
                else:
                    hb = hpool.tile([P, KT, H], bf16, tag=f"hb{b}", name=f"hb{b}")
                    nc.sync.dma_start(hb, hid[b].rearrange("(k p) h -> p k h", p=P))
                    hbs[b] = hb

            def hb_slice(b, k, lo, hi):
                if b < 2:
                    return hbs[b][k][:, lo:hi]
                return hbs[b][:, k, lo:hi]

            def load_w(ws, wdram, k, nm):
                ws[k] = wpool.tile([P, H], bf16, tag=f"{nm}{k}", name=f"{nm}{k}")
                nc.sync.dma_start(ws[k], wdram[k * P : (k + 1) * P, :])

            load_hb(0)
            load_hb(1)
            for k in range(KH):
                load_w(w1ks, w1, k, "w1k")
            load_hb(2)
            load_hb(3)
            nc.sync.dma_start(mtn_sb[:, 2 * KT * S :], mtn[:, 2 * KT * S :])
            for k in range(KH):
                load_w(w2ks, w2, k, "w2k")
            load_hb(4)
            load_hb(5)
            load_hb(6)
            load_hb(7)

            xts = [xtpool.tile([P, R], bf16, tag=f"xt{k}", name=f"xt{k}") for k in range(KH)]
            y1s = [ypool.tile([P, R], bf16, tag=f"y1_{m}", name=f"y1_{m}") for m in range(KH)]
            y2s = [ypool.tile([P, R], bf16, tag=f"y2_{m}", name=f"y2_{m}") for m in range(KH)]
            logits = ypool.tile([1, R], f32, tag="logits")
            pred = ypool.tile([1, R], f32, tag="pred")

            C0 = 512          # pooling psum chunk 0: cols [0, 512)
            C1 = H - C0       # chunk 1: cols [512, 768)

            def pool(b):
                pp0 = ps.tile([P, C0], f32, tag="ps")
                pp1 = ps.tile([P, C1], f32, tag="ps")
                for k in range(KT):
                    mt = mtn_sb[:, (b * KT + k) * S : (b * KT + k + 1) * S]
                    nc.tensor.matmul(
                        pp0, lhsT=mt, rhs=hb_slice(b, k, 0, C0),
                        start=(k == 0), stop=(k == KT - 1),
                    )
                    nc.tensor.matmul(
                        pp1, lhsT=mt, rhs=hb_slice(b, k, C0, H),
                        start=(k == 0), stop=(k == KT - 1),
                    )
                # evacuate psum * inv -> bf16 pooled, in transpose
                # consumption order (first 128 cols unblock transpose m0)
                pooled = small.tile([P, H], bf16, tag="pooled")
                ib = inv_sb[:, b : b + 1]
                nc.vector.tensor_tensor(
                    pooled[:, 0:P], pp0[:, 0:P], ib.to_broadcast((P, P)), OP.mult
                )
                nc.vector.tensor_tensor(
                    pooled[:, P:C0], pp0[:, P:C0],
                    ib.to_broadcast((P, C0 - P)), OP.mult,
                )
                nc.vector.tensor_tensor(
                    pooled[:, C0:H], pp1[:, 0:C1],
                    ib.to_broadcast((P, C1)), OP.mult,
                )
                for m in range(KH):
                    trp = ps.tile([P, P], bf16, tag="ps")
                    nc.tensor.transpose(trp, pooled[:, m * P : (m + 1) * P], ident_sb)
                    nc.vector.tensor_copy(xts[m][:, b * S : (b + 1) * S], trp)

            def fc(wks, b_sb, xs, outs, rc, func):
                for m in range(KH):
                    pt = ps.tile([P, RC], f32, tag="ps")
                    for k in range(KH):
                        nc.tensor.matmul(
                            pt,
                            lhsT=wks[k][:, m * P : (m + 1) * P],
                            rhs=xs[k][:, rc * RC : (rc + 1) * RC],
                            start=(k == 0),
                            stop=(k == KH - 1),
                        )
                    nc.scalar.activation(
                        outs[m][:, rc * RC : (rc + 1) * RC],
                        pt,
                        func,
                        bias=b_sb[:, m : m + 1],
                    )

            def fc3mm(rc):
                ptl = ps.tile([1, RC], f32, tag="ps")
                for k in range(KH):
                    nc.tensor.matmul(
                        ptl,
                        lhsT=w3_sb[:, k : k + 1],
                        rhs=y2s[k][:, rc * RC : (rc + 1) * RC],
                        start=(k == 0),
                        stop=(k == KH - 1),
                    )
                nc.vector.tensor_copy(logits[:, rc * RC : (rc + 1) * RC], ptl)

            def sig(rc):
                nc.scalar.activation(
                    pred[:, rc * RC : (rc + 1) * RC],
                    logits[:, rc * RC : (rc + 1) * RC],
                    FT.Sigmoid,
                    bias=b3_sb,
                )
                nc.sync.dma_start(
                    out.rearrange("b s -> (b s)")[rc * RC : (rc + 1) * RC],
                    pred[:, rc * RC : (rc + 1) * RC],
                )

            G = FT.Gelu
            pool(0)
            pool(1)
            fc(w1ks, b1_sb, xts, y1s, 0, G)
            pool(2)
            pool(3)
            fc(w1ks, b1_sb, xts, y1s, 1, G)
            fc(w2ks, b2_sb, y1s, y2s, 0, G)
            fc3mm(0)
            pool(4)
            pool(5)
            fc(w1ks, b1_sb, xts, y1s, 2, G)
            fc(w2ks, b2_sb, y1s, y2s, 1, G)
            fc3mm(1)
            fc(w2ks, b2_sb, y1s, y2s, 2, G)
            fc3mm(2)
            pool(6)
            pool(7)
            # deferred sigmoids: one gelu->sigmoid table switch, hidden
            # under pool(6/7) matmuls on PE
            sig(0)
            sig(1)
            sig(2)
            fc(w1ks, b1_sb, xts, y1s, 3, G)
            fc(w2ks, b2_sb, y1s, y2s, 3, G)
            fc3mm(3)
            sig(3)

    nc.compile()
    return nc


def _get_program():
    if "nc" not in _CACHE:
        _CACHE["nc"] = _build_program()
    return _CACHE["nc"]


def _cpack(sid_shard, b1, b2, b3, w3):
    """Per-core packed constants: bf16 (identity for PE transpose, w3) and
    f32 (inv = 1/max(count,1), biases). Plus the packed one-hot matrix."""
    oh = (sid_shard[:, :, None] == np.arange(S, dtype=np.int32)[None, None, :])
    counts = oh.sum(axis=1).astype(np.float32)          # [BL, S]
    inv = 1.0 / np.maximum(counts, 1.0)                 # [BL, S]
    mtn = np.ascontiguousarray(
        oh.reshape(BL, KT, P, S).transpose(2, 0, 1, 3).reshape(P, MTC)
    ).astype(BF16)
    ch = np.zeros((P, CH_COLS), dtype=BF16)
    ch[:, 0:P] = np.eye(P, dtype=np.float32)
    ch[:, P : P + KH] = np.asarray(w3, np.float32).reshape(KH, P).T
    cf = np.zeros((P, CF_COLS), dtype=np.float32)
    cf[:, 0:BL] = inv.T
    cf[:, BL : BL + KH] = np.asarray(b1, np.float32).reshape(KH, P).T
    cf[:, BL + KH : BL + 2 * KH] = np.asarray(b2, np.float32).reshape(KH, P).T
    cf[0, BL + 2 * KH] = np.float32(np.asarray(b3).reshape(-1)[0])
    return mtn, ch, cf


def make_in_maps(hidden, statements_ids, w1, b1, w2, b2, w3, b3):
    hidden = np.asarray(hidden, dtype=np.float32).astype(BF16)
    sid = np.asarray(statements_ids, dtype=np.int32)
    w1 = np.ascontiguousarray(np.asarray(w1, dtype=np.float32).astype(BF16))
    w2 = np.ascontiguousarray(np.asarray(w2, dtype=np.float32).astype(BF16))
    in_maps = []
    for c in range(N_CORES):
        mtn, ch, cf = _cpack(sid[c * BL : (c + 1) * BL], b1, b2, b3, w3)
        in_maps.append(
            {
                "hidden": np.ascontiguousarray(hidden[c * BL : (c + 1) * BL]),
                "mtn": mtn,
                "w1": w1,
                "w2": w2,
                "cpack_h": ch,
                "cpack_f": cf,
            }
        )
    return in_maps


def kernel(hidden, statements_ids, w1, b1, w2, b2, w3, b3, **kwargs):
    nc = _get_program()
    in_maps = make_in_maps(hidden, statements_ids, w1, b1, w2, b2, w3, b3)
    trace = bool(int(os.environ.get("KERNEL_TRACE", "0")))
    res = bass_utils.run_bass_kernel_spmd(
        nc, in_maps, core_ids=list(range(N_CORES)), trace=trace
    )
    _CACHE["last_results"] = res
    out = np.concatenate([res.results[c]["out"] for c in range(N_CORES)], axis=0)
    return out.astype(np.float32)


# revision 5
# speedup vs baseline: 1.1357x; 1.0701x over previous
"""Trainium2 Bass kernel: per-batch segment-mean pooling + 3-layer MLP.

Reference computation (B=64, T=512, H=768, S=128):
  pooled[b,s,:] = mean over t of hidden[b,t,:] where statements_ids[b,t]==s
  x = gelu(pooled @ w1 + b1); x = gelu(x @ w2 + b2)
  out[b,s] = sigmoid(x @ w3 + b3)

Distribution: data-parallel over batch across 8 NeuronCores (8 batches per
core); MLP weights replicated.

v2 design notes (PE streams at N cols/cycle regardless of dtype, so the
wins are bandwidth + gap-closing, not matmul cycles):
  - All tensor streams in bf16: halves HBM traffic (9.75 MB/core vs 17.3)
    so the pooling phase is no longer DMA-paced.
  - The one-hot matrix MT[t,s] (exact 0/1 values in bf16) is built on the
    host and DMA'd in: removes the iota/is_equal/cast DVE chain from the
    critical path at startup.
  - counts -> inv = 1/max(counts,1) computed on host (f32, exact): kills
    the padded ones-columns, the counts matmul and the max/reciprocal
    chain; pooling psum is evacuated with a single f32-psum * inv
    broadcast multiply (DVE) per chunk, writing bf16.
  - All 8 hidden batches are SBUF-resident (bf16 halves footprint) so
    every DMA is issued up front and streams at full rate.
  - Sigmoids are deferred and batched (2 ACT table switches instead of 8;
    each 1.28us): rc0-2 sigmoid mid-run under pool(6/7), only rc3's
    sigmoid (+4KB out DMA) sits on the tail behind the last gelu.
"""

import os
import sys

sys.path.insert(0, "/opt/trn_rl_repo")

import ml_dtypes
import numpy as np

import concourse.bass as bass
import concourse.mybir as mybir
import concourse.tile as tile
from concourse import bacc, bass_utils

B, T, H, S = 64, 512, 768, 128
N_CORES = 8
BL = B // N_CORES  # local batches per core
P = 128
KT = T // P        # t-tiles per batch
KH = H // P        # h-tiles
R = BL * S         # MLP rows per core
RC = 2 * S         # moving-dim chunk (2 batches)
NRC = R // RC
MTC = BL * KT * S  # packed one-hot columns
CH_COLS = P + KH           # bf16 packed consts: ident | w3
CF_COLS = BL + 2 * KH + 1  # f32 packed consts: inv | b1 | b2 | b3

BF16 = ml_dtypes.bfloat16

_CACHE: dict = {}


def _build_program():
    f32, bf16 = mybir.dt.float32, mybir.dt.bfloat16
    FT = mybir.ActivationFunctionType
    OP = mybir.AluOpType

    nc = bacc.Bacc("TRN2", target_bir_lowering=False, debug=False)
    hid = nc.dram_tensor("hidden", [BL, T, H], bf16, kind="ExternalInput").ap()
    mtn = nc.dram_tensor("mtn", [P, MTC], bf16, kind="ExternalInput").ap()
    w1 = nc.dram_tensor("w1", [H, H], bf16, kind="ExternalInput").ap()
    w2 = nc.dram_tensor("w2", [H, H], bf16, kind="ExternalInput").ap()
    cpack_h = nc.dram_tensor("cpack_h", [P, CH_COLS], bf16, kind="ExternalInput").ap()
    cpack_f = nc.dram_tensor("cpack_f", [P, CF_COLS], f32, kind="ExternalInput").ap()
    out = nc.dram_tensor("out", [BL, S], f32, kind="ExternalOutput").ap()

    with tile.TileContext(nc) as tc:
        with (
            tc.tile_pool(name="consts", bufs=1) as consts,
            tc.tile_pool(name="wpool", bufs=1) as wpool,
            tc.tile_pool(name="hpool", bufs=1) as hpool,
            tc.tile_pool(name="small", bufs=3) as small,
            tc.tile_pool(name="xtpool", bufs=1) as xtpool,
            tc.tile_pool(name="ypool", bufs=1) as ypool,
            tc.tile_pool(name="ps", bufs=8, space="PSUM") as ps,
        ):
            # ---- constants first (small, unblock nothing heavy) ----
            cph_sb = consts.tile([P, CH_COLS], bf16)
            nc.sync.dma_start(cph_sb, cpack_h)
            cpf_sb = consts.tile([P, CF_COLS], f32)
            nc.sync.dma_start(cpf_sb, cpack_f)
            ident_sb = cph_sb[:, 0:P]
            w3_sb = cph_sb[:, P : P + KH]
            inv_sb = cpf_sb[:, 0:BL]
            b1_sb = cpf_sb[:, BL : BL + KH]
            b2_sb = cpf_sb[:, BL + KH : BL + 2 * KH]
            b3_sb = cpf_sb[0:1, BL + 2 * KH : BL + 2 * KH + 1]

            # one-hot pack, DMA'd in 2-batch slices just ahead of their pool()
            mtn_sb = consts.tile([P, MTC], bf16)
            MB = 2 * KT * S  # cols per 2-batch slice

            def load_mtn(i):
                nc.sync.dma_start(mtn_sb[:, i * MB : (i + 1) * MB],
                                  mtn[:, i * MB : (i + 1) * MB])

            load_mtn(0)

            # ---- hidden + weight streaming; everything SBUF-resident so
            # DMA free-runs; issue order == consumption order ----
            hbs = [None] * BL
            w1ks = [None] * KH
            w2ks = [None] * KH

            def load_hb(b):
                if b < 2:
                    # first two batches arrive per k-chunk so pooling can
                    # start on the first 0.2 MB
                    tiles = []
                    for k in range(KT):
                        t = hpool.tile([P, H], bf16, tag=f"hb{b}k{k}", name=f"hb{b}k{k}")
                        nc.sync.dma_start(t, hid[b, k * P : (k + 1) * P, :])
                        tiles.append(t)
                    hbs[b] = tiles
                else:
                    hb = hpool.tile([P, KT, H], bf16, tag=f"hb{b}", name=f"hb{b}")
                    nc.sync.dma_start(hb, hid[b].rearrange("(k p) h -> p k h", p=P))
                    hbs[b] = hb

            def hb_slice(b, k, lo, hi):
                if b < 2:
                    return hbs[b][k][:, lo:hi]
                return hbs[b][:, k, lo:hi]

            def load_w(ws, wdram, k, nm):
                ws[k] = wpool.tile([P, H], bf16, tag=f"{nm}{k}", name=f"{nm}{k}")
                nc.sync.dma_start(ws[k], wdram[k * P : (k + 1) * P, :])

            load_hb(0)
            load_hb(1)
            for k in range(KH):
                load_w(w1ks, w1, k, "w1k")
            load_mtn(1)
            load_hb(2)
            load_hb(3)
            for k in range(3):
                load_w(w2ks, w2, k, "w2k")
            load_mtn(2)
            load_hb(4)
            load_hb(5)
            for k in range(3, KH):
                load_w(w2ks, w2, k, "w2k")
            load_mtn(3)
            load_hb(6)
            load_hb(7)

            xts = [xtpool.tile([P, R], bf16, tag=f"xt{k}", name=f"xt{k}") for k in range(KH)]
            y1s = [ypool.tile([P, R], bf16, tag=f"y1_{m}", name=f"y1_{m}") for m in range(KH)]
            y2s = [ypool.tile([P, R], bf16, tag=f"y2_{m}", name=f"y2_{m}") for m in range(KH)]
            logits = ypool.tile([1, R], f32, tag="logits")
            pred = ypool.tile([1, R], f32, tag="pred")

            C0 = 512          # pooling psum chunk 0: cols [0, 512)
            C1 = H - C0       # chunk 1: cols [512, 768)

            def pool(b):
                pp0 = ps.tile([P, C0], f32, tag="ps")
                pp1 = ps.tile([P, C1], f32, tag="ps")
                for k in range(KT):
                    # short MM first, long MM second: the next k's
                    # LDWEIGHTS fully hides under the 512-col stream
                    mt = mtn_sb[:, (b * KT + k) * S : (b * KT + k + 1) * S]
                    nc.tensor.matmul(
                        pp1, lhsT=mt, rhs=hb_slice(b, k, C0, H),
                        start=(k == 0), stop=(k == KT - 1),
                    )
                    nc.tensor.matmul(
                        pp0, lhsT=mt, rhs=hb_slice(b, k, 0, C0),
                        start=(k == 0), stop=(k == KT - 1),
                    )
                # evacuate psum * inv -> bf16 pooled, in transpose
                # consumption order (first 128 cols unblock transpose m0)
                pooled = small.tile([P, H], bf16, tag="pooled")
                ib = inv_sb[:, b : b + 1]
                nc.vector.tensor_tensor(
                    pooled[:, 0:P], pp0[:, 0:P], ib.to_broadcast((P, P)), OP.mult
                )
                nc.vector.tensor_tensor(
                    pooled[:, P:C0], pp0[:, P:C0],
                    ib.to_broadcast((P, C0 - P)), OP.mult,
                )
                nc.vector.tensor_tensor(
                    pooled[:, C0:H], pp1[:, 0:C1],
                    ib.to_broadcast((P, C1)), OP.mult,
                )
                for m in range(KH):
                    trp = ps.tile([P, P], bf16, tag="ps")
                    nc.tensor.transpose(trp, pooled[:, m * P : (m + 1) * P], ident_sb)
                    nc.vector.tensor_copy(xts[m][:, b * S : (b + 1) * S], trp)

            def fc(wks, b_sb, xs, outs, rc, func):
                for m in range(KH):
                    pt = ps.tile([P, RC], f32, tag="ps")
                    for k in range(KH):
                        nc.tensor.matmul(
                            pt,
                            lhsT=wks[k][:, m * P : (m + 1) * P],
                            rhs=xs[k][:, rc * RC : (rc + 1) * RC],
                            start=(k == 0),
                            stop=(k == KH - 1),
                        )
                    nc.scalar.activation(
                        outs[m][:, rc * RC : (rc + 1) * RC],
                        pt,
                        func,
                        bias=b_sb[:, m : m + 1],
                    )

            def fc3mm(rc):
                ptl = ps.tile([1, RC], f32, tag="ps")
                for k in range(KH):
                    nc.tensor.matmul(
                        ptl,
                        lhsT=w3_sb[:, k : k + 1],
                        rhs=y2s[k][:, rc * RC : (rc + 1) * RC],
                        start=(k == 0),
                        stop=(k == KH - 1),
                    )
                nc.vector.tensor_copy(logits[:, rc * RC : (rc + 1) * RC], ptl)

            def sig(rc):
                nc.scalar.activation(
                    pred[:, rc * RC : (rc + 1) * RC],
                    logits[:, rc * RC : (rc + 1) * RC],
                    FT.Sigmoid,
                    bias=b3_sb,
                )
                nc.sync.dma_start(
                    out.rearrange("b s -> (b s)")[rc * RC : (rc + 1) * RC],
                    pred[:, rc * RC : (rc + 1) * RC],
                )

            G = FT.Gelu
            pool(0)
            pool(1)
            fc(w1ks, b1_sb, xts, y1s, 0, G)
            pool(2)
            pool(3)
            fc(w1ks, b1_sb, xts, y1s, 1, G)
            fc(w2ks, b2_sb, y1s, y2s, 0, G)
            fc3mm(0)
            pool(4)
            pool(5)
            fc(w1ks, b1_sb, xts, y1s, 2, G)
            fc(w2ks, b2_sb, y1s, y2s, 1, G)
            fc3mm(1)
            fc(w2ks, b2_sb, y1s, y2s, 2, G)
            fc3mm(2)
            pool(6)
            pool(7)
            # deferred sigmoids: one gelu->sigmoid table switch, hidden
            # under pool(6/7) matmuls on PE
            sig(0)
            sig(1)
            sig(2)
            fc(w1ks, b1_sb, xts, y1s, 3, G)
            fc(w2ks, b2_sb, y1s, y2s, 3, G)
            fc3mm(3)
            sig(3)

    nc.compile()
    return nc


def _get_program():
    if "nc" not in _CACHE:
        _CACHE["nc"] = _build_program()
    return _CACHE["nc"]


def _cpack(sid_shard, b1, b2, b3, w3):
    """Per-core packed constants: bf16 (identity for PE transpose, w3) and
    f32 (inv = 1/max(count,1), biases). Plus the packed one-hot matrix."""
    oh = (sid_shard[:, :, None] == np.arange(S, dtype=np.int32)[None, None, :])
    counts = oh.sum(axis=1).astype(np.float32)          # [BL, S]
    inv = 1.0 / np.maximum(counts, 1.0)                 # [BL, S]
    mtn = np.ascontiguousarray(
        oh.reshape(BL, KT, P, S).transpose(2, 0, 1, 3).reshape(P, MTC)
    ).astype(BF16)
    ch = np.zeros((P, CH_COLS), dtype=BF16)
    ch[:, 0:P] = np.eye(P, dtype=np.float32)
    ch[:, P : P + KH] = np.asarray(w3, np.float32).reshape(KH, P).T
    cf = np.zeros((P, CF_COLS), dtype=np.float32)
    cf[:, 0:BL] = inv.T
    cf[:, BL : BL + KH] = np.asarray(b1, np.float32).reshape(KH, P).T
    cf[:, BL + KH : BL + 2 * KH] = np.asarray(b2, np.float32).reshape(KH, P).T
    cf[0, BL + 2 * KH] = np.float32(np.asarray(b3).reshape(-1)[0])
    return mtn, ch, cf


def make_in_maps(hidden, statements_ids, w1, b1, w2, b2, w3, b3):
    hidden = np.asarray(hidden, dtype=np.float32).astype(BF16)
    sid = np.asarray(statements_ids, dtype=np.int32)
    w1 = np.ascontiguousarray(np.asarray(w1, dtype=np.float32).astype(BF16))
    w2 = np.ascontiguousarray(np.asarray(w2, dtype=np.float32).astype(BF16))
    in_maps = []
    for c in range(N_CORES):
        mtn, ch, cf = _cpack(sid[c * BL : (c + 1) * BL], b1, b2, b3, w3)
        in_maps.append(
            {
                "hidden": np.ascontiguousarray(hidden[c * BL : (c + 1) * BL]),
                "mtn": mtn,
                "w1": w1,
                "w2": w2,
                "cpack_h": ch,
                "cpack_f": cf,
            }
        )
    return in_maps


def kernel(hidden, statements_ids, w1, b1, w2, b2, w3, b3, **kwargs):
    nc = _get_program()
    in_maps = make_in_maps(hidden, statements_ids, w1, b1, w2, b2, w3, b3)
    trace = bool(int(os.environ.get("KERNEL_TRACE", "0")))
    res = bass_utils.run_bass_kernel_spmd(
        nc, in_maps, core_ids=list(range(N_CORES)), trace=trace
    )
    _CACHE["last_results"] = res
    out = np.concatenate([res.results[c]["out"] for c in range(N_CORES)], axis=0)
    return out.astype(np.float32)
